# revision 7
# baseline (speedup 1.0000x reference)
"""Trainium kernel for nn_HeterogeneousGAT: 2-layer heterogeneous GAT.

Device (8 NeuronCores, SPMD, row-sharded): the five per-node MLP stacks of
each operation_layer (the dominant ~48 GFLOP), in feature-major layout.
Host: graph gathers/scatter-adds, global-softmax attention (resource_layer),
pooling and actor/critic head.
"""
import os
import sys

sys.path.insert(0, "/opt/trn_rl_repo")

import numpy as np

import bass_rust
import concourse.bass as bass
import concourse.mybir as mybir
from concourse.bass_utils import run_bass_kernel_spmd
from concourse.tile import TileContext, ScopedClock

F = mybir.ActivationFunctionType
A = mybir.AluOpType
DT = mybir.dt.float32

N_OPS, N_RES, N_EDGES, N_ACT = 131072, 16384, 1048576, 4096
EMB, HID = 8, 128
N_CORES = 8
ROWS = N_OPS // N_CORES  # 16384 rows per core
CHUNK = 512
N_CHUNKS = ROWS // CHUNK

_exec_times = []

# ---------------------------------------------------------------- tile patch


def _drain_and_barrier_split(self, tick_clock, wait_clock):
    nc = self.nc
    collector = nc.sync.nop(nofuse=True)
    wait_clock.add_sem_waits(
        collector.ins, ScopedClock({None: tick_clock.global_clock})
    )
    si = collector.ins.sync_info
    waits = list(si.on_wait) if si is not None else []
    if waits:
        collector.ins.sync_info = bass_rust.SyncInfo(
            on_wait=[waits[0]], on_update=[]
        )
        for w in waits[1:]:
            n = nc.sync.nop(nofuse=True)
            n.ins.sync_info = bass_rust.SyncInfo(on_wait=[w], on_update=[])
    nc.sync.drain()
    nc.all_engine_barrier()
    assert self.sems is not None
    popped = nc._tile_sem_poison_stack.pop()
    assert popped is self._sem_poison
    nc.clear_and_free_semaphores(list(self.sems.allocated().values()))
    nc.all_engine_barrier()


TileContext._drain_and_barrier = _drain_and_barrier_split


def _split_waits(nc):
    """This walrus accepts one sync-wait per instruction (2 for
    EventSemaphore); move extras onto same-engine NOPs placed before."""
    n_new = 0
    for fn in nc.m.functions:
        for bb in fn.blocks:
            insns = bb.instructions
            out = []
            for inst in insns:
                si = inst.sync_info
                cap = 2 if isinstance(inst, mybir.InstEventSemaphore) else 1
                if si is not None and len(si.on_wait) > cap:
                    waits = list(si.on_wait)
                    keep, extra = waits[:cap], waits[cap:]
                    for w in extra:
                        n_new += 1
                        out.append(
                            mybir.InstNoOp(
                                name=f"waitnop-{n_new}-{inst.name}",
                                engine=inst.engine,
                                ins=[],
                                outs=[],
                                sync_info=bass_rust.SyncInfo(
                                    on_wait=[w], on_update=[]
                                ),
                            )
                        )
                    inst.sync_info = bass_rust.SyncInfo(
                        on_wait=keep, on_update=list(si.on_update)
                    )
                out.append(inst)
            if len(out) != len(insns):
                insns[:] = out
    return n_new


# ---------------------------------------------------------------- device nc

MLPS = ["pred", "succ", "res", "same"]  # comb concat order: preds,succs,aggm,same


def _build_nc():
    nc = bass.Bass()
    x_all = nc.declare_dram_parameter("x_all", [32, ROWS], DT, isOutput=False)
    wt = {}
    for m in MLPS + ["comb"]:
        if m == "comb":
            for j in range(4):
                wt[m, f"w1_{j}"] = nc.declare_dram_parameter(
                    f"{m}_w1_{j}", [8, HID], DT, isOutput=False
                )
        else:
            wt[m, "w1"] = nc.declare_dram_parameter(f"{m}_w1", [8, HID], DT, isOutput=False)
        wt[m, "b1"] = nc.declare_dram_parameter(f"{m}_b1", [HID, 1], DT, isOutput=False)
        wt[m, "w2"] = nc.declare_dram_parameter(f"{m}_w2", [HID, HID], DT, isOutput=False)
        wt[m, "b2"] = nc.declare_dram_parameter(f"{m}_b2", [HID, 1], DT, isOutput=False)
        wt[m, "w3"] = nc.declare_dram_parameter(f"{m}_w3", [HID, 8], DT, isOutput=False)
        wt[m, "b3"] = nc.declare_dram_parameter(f"{m}_b3", [8, 1], DT, isOutput=False)
    out = nc.declare_dram_parameter("out", [8, ROWS], DT, isOutput=True)

    with TileContext(nc) as tc:
        with (
            tc.tile_pool(name="xin", bufs=8) as xin,
            tc.tile_pool(name="wp", bufs=1) as wp,
            tc.tile_pool(name="hb", bufs=6) as hb,
            tc.tile_pool(name="ccp", bufs=3) as ccp,
            tc.tile_pool(name="op", bufs=3) as op,
            tc.tile_pool(name="ps", bufs=2, space="PSUM") as ps,
            tc.tile_pool(name="ps3", bufs=2, space="PSUM") as ps3,
        ):
            w = {}
            for m in MLPS + ["comb"]:
                keys = ["b1", "w2", "b2", "w3", "b3"] + (
                    [f"w1_{j}" for j in range(4)] if m == "comb" else ["w1"]
                )
                for k in keys:
                    t = wp.tile(list(wt[m, k].shape), DT, tag=f"{m}{k}")
                    nc.gpsimd.dma_start(out=t[:], in_=wt[m, k][:])
                    w[m, k] = t

            def hidden(p_ap, b_tile, tag):
                ex = hb.tile([HID, CHUNK], DT, tag=f"ex")
                nc.scalar.activation(ex[:], p_ap, F.Exp, bias=b_tile[:], scale=1.0)
                r = hb.tile([HID, CHUNK], DT, tag=f"r")
                nc.vector.tensor_scalar(r[:], p_ap, b_tile[:], 0.0, A.add, A.max)
                h = hb.tile([HID, CHUNK], DT, tag=f"h{tag}")
                nc.vector.scalar_tensor_tensor(h[:], ex[:], 1.0, r[:], A.min, A.add)
                return h

            def mlp(m, x_ap, out_ap):
                p1 = ps.tile([HID, CHUNK], DT, tag="p1")
                if m == "comb":
                    for j in range(4):
                        nc.tensor.matmul(
                            p1[:],
                            w[m, f"w1_{j}"][:],
                            x_ap[j][:],
                            start=(j == 0),
                            stop=(j == 3),
                        )
                else:
                    nc.tensor.matmul(p1[:], w[m, "w1"][:], x_ap, start=True, stop=True)
                h1 = hidden(p1[:], w[m, "b1"], "1")
                p2 = ps.tile([HID, CHUNK], DT, tag="p2")
                nc.tensor.matmul(p2[:], w[m, "w2"][:], h1[:], start=True, stop=True)
                h2 = hidden(p2[:], w[m, "b2"], "2")
                p3 = ps3.tile([8, CHUNK], DT, tag="p3")
                nc.tensor.matmul(p3[:], w[m, "w3"][:], h2[:], start=True, stop=True)
                nc.vector.tensor_scalar(out_ap, p3[:], w[m, "b3"][:], None, A.add)

            for c in range(N_CHUNKS):
                sl = slice(c * CHUNK, (c + 1) * CHUNK)
                ys = []
                for mi, m in enumerate(MLPS):
                    xt = xin.tile([8, CHUNK], DT, tag="xin")
                    nc.gpsimd.dma_start(
                        out=xt[:], in_=x_all[8 * mi : 8 * mi + 8, sl]
                    )
                    y = ccp.tile([8, CHUNK], DT, tag=f"y{m}")
                    mlp(m, xt[:], y[:])
                    ys.append(y)
                ot = op.tile([8, CHUNK], DT, tag="ot")
                mlp("comb", ys, ot[:])
                nc.gpsimd.dma_start(out=out[:, sl], in_=ot[:])

    _split_waits(nc)
    return nc


_NC = None


def _get_nc():
    global _NC
    if _NC is None:
        _NC = _build_nc()
    return _NC


# ---------------------------------------------------------------- host math


def _elu(x):
    return np.where(x > 0, x, np.expm1(np.minimum(x, 0.0)))


def _lrelu(x):
    return np.where(x > 0, x, 0.2 * x)


def _mlp_np(p, x, act):
    h = act(x @ p["l1"]["w"] + p["l1"]["b"])
    h = act(h @ p["l2"]["w"] + p["l2"]["b"])
    return h @ p["l3"]["w"] + p["l3"]["b"]


def _seg_sum(idx, vals, n):
    out = np.empty((n, vals.shape[1]), np.float32)
    for f in range(vals.shape[1]):
        out[:, f] = np.bincount(idx, weights=vals[:, f], minlength=n)
    return out


def _resource_layer(p, resources, operations, req):
    r = resources @ p["Wr"]
    o = operations @ p["Wo"]
    ops_e = o[req[0]]
    res_e = r[req[1]]
    a_self = _lrelu(np.concatenate([r, r], -1) @ p["a_self"])
    a_cross = _lrelu(res_e @ p["a"][:EMB] + ops_e @ p["a"][EMB:])
    cat = np.concatenate([a_self, a_cross], 0)
    ex = np.exp(cat - cat.max())
    norm = ex / ex.sum()
    ns, ncr = norm[: r.shape[0]], norm[r.shape[0] :]
    agg = _seg_sum(req[1], (ncr * ops_e).astype(np.float32), r.shape[0])
    return _elu(ns * r + agg).astype(np.float32)


def _np(x):
    return np.asarray(x, dtype=np.float32)


def _op_layer_inputs(operations, resources, prec, req):
    n = operations.shape[0]
    src, dst = prec[0], prec[1]
    agg_mach = _seg_sum(req[0], resources[req[1]], n)
    cnt_in = np.maximum(np.bincount(dst, minlength=n), 1.0)[:, None]
    cnt_out = np.maximum(np.bincount(src, minlength=n), 1.0)[:, None]
    pred_mean = _seg_sum(dst, operations[src], n) / cnt_in
    succ_mean = _seg_sum(src, operations[dst], n) / cnt_out
    return pred_mean.astype(np.float32), succ_mean.astype(np.float32), agg_mach


def _pad8(xT):
    if xT.shape[0] == 8:
        return np.ascontiguousarray(xT)
    out = np.zeros((8, xT.shape[1]), np.float32)
    out[: xT.shape[0]] = xT
    return out


def _wmap(p):
    """Flatten one op-layer's params into device weight arrays, folding the
    (elu+1) hidden shift into l2/l3 biases: device h' = elu(z)+1, so
    b' = b - colsum(W)."""
    out = {}
    for m in MLPS + ["comb"]:
        q = p[m]
        w1 = _np(q["l1"]["w"])
        if m == "comb":
            for j in range(4):
                out[f"{m}_w1_{j}"] = np.ascontiguousarray(w1[8 * j : 8 * j + 8])
        else:
            w1p = np.zeros((8, HID), np.float32)
            w1p[: w1.shape[0]] = w1
            out[f"{m}_w1"] = w1p
        out[f"{m}_b1"] = _np(q["l1"]["b"]).reshape(HID, 1)
        w2 = _np(q["l2"]["w"])
        out[f"{m}_w2"] = w2
        out[f"{m}_b2"] = (_np(q["l2"]["b"]) - w2.sum(0)).reshape(HID, 1)
        w3 = _np(q["l3"]["w"])
        out[f"{m}_w3"] = w3
        out[f"{m}_b3"] = (_np(q["l3"]["b"]) - w3.sum(0)).reshape(8, 1)
    return out


def _run_op_layer_device(p, operations, resources, prec, req):
    pred_mean, succ_mean, agg_mach = _op_layer_inputs(
        operations, resources, prec, req
    )
    wmap = _wmap(p)
    in_maps = []
    for k in range(N_CORES):
        sl = slice(k * ROWS, (k + 1) * ROWS)
        m = dict(wmap)
        m["x_all"] = np.concatenate(
            [
                _pad8(pred_mean[sl].T),
                _pad8(succ_mean[sl].T),
                _pad8(agg_mach[sl].T),
                _pad8(operations[sl].T),
            ],
            axis=0,
        )
        in_maps.append(m)
    nc = _get_nc()
    trace = bool(int(os.environ.get("BASS_KERNEL_TRACE", "0")))
    res = run_bass_kernel_spmd(nc, in_maps, list(range(N_CORES)), trace=trace)
    if trace:
        _exec_times.append(res.exec_time_ns)
    out = np.concatenate(
        [res.results[k]["out"].T for k in range(N_CORES)], axis=0
    ).astype(np.float32)
    out[0] = 0.0
    out[-1] = 0.0
    return out


def kernel(operations, resources, precedence_edges, requirement_edges, actions, t, params):
    operations = _np(operations)
    resources = _np(resources)
    prec = np.asarray(precedence_edges)
    req = np.asarray(requirement_edges)
    actions = np.asarray(actions)
    def _conv(v):
        return {k: _conv(x) for k, x in v.items()} if isinstance(v, dict) else _np(v)

    pp = _conv(params)

    ops, res = operations, resources
    for l in range(2):
        res = _resource_layer(pp[f"res{l}"], res, ops, req)
        ops = _run_op_layer_device(pp[f"op{l}"], ops, res, prec, req)

    graph_state = np.concatenate([ops.mean(0), res.mean(0)], -1)
    state_value = _mlp_np(pp["critic"], graph_state, np.tanh)
    act_in = np.concatenate(
        [
            ops[actions[:, 0]],
            res[actions[:, 1]],
            np.broadcast_to(graph_state, (actions.shape[0], 2 * EMB)),
        ],
        -1,
    ).astype(np.float32)
    logits = _mlp_np(pp["actor"], act_in, np.tanh)
    ex = np.exp(logits - logits.max())
    probs = (ex / ex.sum()).astype(np.float32)
    return probs, np.asarray(state_value, dtype=np.float32)


# revision 10
# speedup vs baseline: 1.6849x; 1.6849x over previous
"""Trainium kernel for nn_HeterogeneousGAT: 2-layer heterogeneous GAT.

Device (8 NeuronCores, SPMD, row-sharded): the five per-node MLP stacks of
each operation_layer (the dominant ~48 GFLOP), in feature-major layout.
Host: graph gathers/scatter-adds, global-softmax attention (resource_layer),
pooling and actor/critic head.
"""
import os
import sys

sys.path.insert(0, "/opt/trn_rl_repo")

import numpy as np
import ml_dtypes

BF16 = ml_dtypes.bfloat16

import bass_rust
import concourse.bass as bass
import concourse.mybir as mybir
from concourse.bass_utils import run_bass_kernel_spmd
from concourse.tile import TileContext, ScopedClock

F = mybir.ActivationFunctionType
A = mybir.AluOpType
DT = mybir.dt.float32
DTB = mybir.dt.bfloat16
DTR = mybir.dt.float32r

N_OPS, N_RES, N_EDGES, N_ACT = 131072, 16384, 1048576, 4096
EMB, HID = 8, 128
N_CORES = 8
ROWS = N_OPS // N_CORES  # 16384 rows per core
CHUNK = 512
N_CHUNKS = ROWS // CHUNK

_exec_times = []

# ---------------------------------------------------------------- tile patch


def _drain_and_barrier_split(self, tick_clock, wait_clock):
    nc = self.nc
    collector = nc.sync.nop(nofuse=True)
    wait_clock.add_sem_waits(
        collector.ins, ScopedClock({None: tick_clock.global_clock})
    )
    si = collector.ins.sync_info
    waits = list(si.on_wait) if si is not None else []
    if waits:
        collector.ins.sync_info = bass_rust.SyncInfo(
            on_wait=[waits[0]], on_update=[]
        )
        for w in waits[1:]:
            n = nc.sync.nop(nofuse=True)
            n.ins.sync_info = bass_rust.SyncInfo(on_wait=[w], on_update=[])
    nc.sync.drain()
    nc.all_engine_barrier()
    assert self.sems is not None
    popped = nc._tile_sem_poison_stack.pop()
    assert popped is self._sem_poison
    nc.clear_and_free_semaphores(list(self.sems.allocated().values()))
    nc.all_engine_barrier()


TileContext._drain_and_barrier = _drain_and_barrier_split


def _split_waits(nc):
    """This walrus accepts one sync-wait per instruction (2 for
    EventSemaphore); move extras onto same-engine NOPs placed before."""
    n_new = 0
    for fn in nc.m.functions:
        for bb in fn.blocks:
            insns = bb.instructions
            out = []
            for inst in insns:
                si = inst.sync_info
                cap = 2 if isinstance(inst, mybir.InstEventSemaphore) else 1
                if si is not None and len(si.on_wait) > cap:
                    waits = list(si.on_wait)
                    keep, extra = waits[:cap], waits[cap:]
                    for w in extra:
                        n_new += 1
                        out.append(
                            mybir.InstNoOp(
                                name=f"waitnop-{n_new}-{inst.name}",
                                engine=inst.engine,
                                ins=[],
                                outs=[],
                                sync_info=bass_rust.SyncInfo(
                                    on_wait=[w], on_update=[]
                                ),
                            )
                        )
                    inst.sync_info = bass_rust.SyncInfo(
                        on_wait=keep, on_update=list(si.on_update)
                    )
                out.append(inst)
            if len(out) != len(insns):
                insns[:] = out
    return n_new


# ---------------------------------------------------------------- device nc

MLPS = ["pred", "succ", "res", "same"]  # comb concat order: preds,succs,aggm,same


def _build_nc():
    nc = bass.Bass()
    x_all = nc.declare_dram_parameter("x_all", [32, ROWS], DT, isOutput=False)
    wt = {}
    for m in MLPS + ["comb"]:
        if m == "comb":
            for j in range(4):
                wt[m, f"w1_{j}"] = nc.declare_dram_parameter(
                    f"{m}_w1_{j}", [8, HID], DT, isOutput=False
                )
        else:
            wt[m, "w1"] = nc.declare_dram_parameter(f"{m}_w1", [8, HID], DT, isOutput=False)
        wt[m, "b1"] = nc.declare_dram_parameter(f"{m}_b1", [HID, 1], DT, isOutput=False)
        wt[m, "w2"] = nc.declare_dram_parameter(f"{m}_w2", [HID, HID], DT, isOutput=False)
        wt[m, "b2"] = nc.declare_dram_parameter(f"{m}_b2", [HID, 1], DT, isOutput=False)
        wt[m, "w3"] = nc.declare_dram_parameter(f"{m}_w3", [HID, 8], DT, isOutput=False)
        wt[m, "b3"] = nc.declare_dram_parameter(f"{m}_b3", [8, 1], DT, isOutput=False)
    out = nc.declare_dram_parameter("out", [8, ROWS], DT, isOutput=True)

    with TileContext(nc) as tc:
        with (
            tc.tile_pool(name="xin", bufs=8) as xin,
            tc.tile_pool(name="wp", bufs=1) as wp,
            tc.tile_pool(name="hb", bufs=6) as hb,
            tc.tile_pool(name="ccp", bufs=3) as ccp,
            tc.tile_pool(name="op", bufs=3) as op,
            tc.tile_pool(name="ps", bufs=2, space="PSUM") as ps,
            tc.tile_pool(name="ps3", bufs=2, space="PSUM") as ps3,
        ):
            w = {}
            for m in MLPS + ["comb"]:
                keys = ["b1", "w2", "b2", "w3", "b3"] + (
                    [f"w1_{j}" for j in range(4)] if m == "comb" else ["w1"]
                )
                for k in keys:
                    dt_k = DT if k.startswith("b") else DTR
                    t = wp.tile(list(wt[m, k].shape), dt_k, tag=f"{m}{k}")
                    nc.gpsimd.dma_start(out=t[:], in_=wt[m, k][:])
                    w[m, k] = t

            def hidden(p_ap, b_tile, tag):
                ex = hb.tile([HID, CHUNK], DT, tag=f"ex")
                nc.scalar.activation(ex[:], p_ap, F.Exp, bias=b_tile[:], scale=1.0)
                r = hb.tile([HID, CHUNK], DT, tag=f"r")
                nc.vector.tensor_scalar(r[:], p_ap, b_tile[:], 0.0, A.add, A.max)
                h = hb.tile([HID, CHUNK], DTR, tag=f"h{tag}")
                nc.vector.scalar_tensor_tensor(h[:], ex[:], 1.0, r[:], A.min, A.add)
                return h

            def mlp(m, x_ap, out_ap):
                p1 = ps.tile([HID, CHUNK], DT, tag="p1")
                if m == "comb":
                    for j in range(4):
                        nc.tensor.matmul(
                            p1[:],
                            w[m, f"w1_{j}"][:],
                            x_ap[j][:],
                            start=(j == 0),
                            stop=(j == 3),
                        )
                else:
                    nc.tensor.matmul(p1[:], w[m, "w1"][:], x_ap, start=True, stop=True)
                h1 = hidden(p1[:], w[m, "b1"], "1")
                p2 = ps.tile([HID, CHUNK], DT, tag="p2")
                nc.tensor.matmul(p2[:], w[m, "w2"][:], h1[:], start=True, stop=True)
                h2 = hidden(p2[:], w[m, "b2"], "2")
                p3 = ps3.tile([8, CHUNK], DT, tag="p3")
                nc.tensor.matmul(p3[:], w[m, "w3"][:], h2[:], start=True, stop=True)
                nc.vector.tensor_scalar(out_ap, p3[:], w[m, "b3"][:], None, A.add)

            for c in range(N_CHUNKS):
                sl = slice(c * CHUNK, (c + 1) * CHUNK)
                ys = []
                for mi, m in enumerate(MLPS):
                    xt = xin.tile([8, CHUNK], DTR, tag="xin")
                    nc.gpsimd.dma_start(
                        out=xt[:], in_=x_all[8 * mi : 8 * mi + 8, sl]
                    )
                    y = ccp.tile([8, CHUNK], DTR, tag=f"y{m}")
                    mlp(m, xt[:], y[:])
                    ys.append(y)
                ot = op.tile([8, CHUNK], DT, tag="ot")
                mlp("comb", ys, ot[:])
                nc.gpsimd.dma_start(out=out[:, sl], in_=ot[:])

    _split_waits(nc)
    return nc


_NC = None


def _get_nc():
    global _NC
    if _NC is None:
        _NC = _build_nc()
    return _NC


# ---------------------------------------------------------------- host math


def _elu(x):
    return np.where(x > 0, x, np.expm1(np.minimum(x, 0.0)))


def _lrelu(x):
    return np.where(x > 0, x, 0.2 * x)


def _mlp_np(p, x, act):
    h = act(x @ p["l1"]["w"] + p["l1"]["b"])
    h = act(h @ p["l2"]["w"] + p["l2"]["b"])
    return h @ p["l3"]["w"] + p["l3"]["b"]


def _seg_sum(idx, vals, n):
    out = np.empty((n, vals.shape[1]), np.float32)
    for f in range(vals.shape[1]):
        out[:, f] = np.bincount(idx, weights=vals[:, f], minlength=n)
    return out


def _resource_layer(p, resources, operations, req):
    r = resources @ p["Wr"]
    o = operations @ p["Wo"]
    ops_e = o[req[0]]
    res_e = r[req[1]]
    a_self = _lrelu(np.concatenate([r, r], -1) @ p["a_self"])
    a_cross = _lrelu(res_e @ p["a"][:EMB] + ops_e @ p["a"][EMB:])
    cat = np.concatenate([a_self, a_cross], 0)
    ex = np.exp(cat - cat.max())
    norm = ex / ex.sum()
    ns, ncr = norm[: r.shape[0]], norm[r.shape[0] :]
    agg = _seg_sum(req[1], (ncr * ops_e).astype(np.float32), r.shape[0])
    return _elu(ns * r + agg).astype(np.float32)


def _np(x):
    return np.asarray(x, dtype=np.float32)


def _op_layer_inputs(operations, resources, prec, req):
    n = operations.shape[0]
    src, dst = prec[0], prec[1]
    agg_mach = _seg_sum(req[0], resources[req[1]], n)
    cnt_in = np.maximum(np.bincount(dst, minlength=n), 1.0)[:, None]
    cnt_out = np.maximum(np.bincount(src, minlength=n), 1.0)[:, None]
    pred_mean = _seg_sum(dst, operations[src], n) / cnt_in
    succ_mean = _seg_sum(src, operations[dst], n) / cnt_out
    return pred_mean.astype(np.float32), succ_mean.astype(np.float32), agg_mach


def _pad8(xT):
    if xT.shape[0] == 8:
        return np.ascontiguousarray(xT)
    out = np.zeros((8, xT.shape[1]), np.float32)
    out[: xT.shape[0]] = xT
    return out


def _wmap(p):
    """Flatten one op-layer's params into device weight arrays, folding the
    (elu+1) hidden shift into l2/l3 biases: device h' = elu(z)+1, so
    b' = b - colsum(W)."""
    out = {}
    for m in MLPS + ["comb"]:
        q = p[m]
        w1 = _np(q["l1"]["w"])
        if m == "comb":
            for j in range(4):
                out[f"{m}_w1_{j}"] = np.ascontiguousarray(w1[8 * j : 8 * j + 8])
        else:
            w1p = np.zeros((8, HID), np.float32)
            w1p[: w1.shape[0]] = w1
            out[f"{m}_w1"] = w1p
        out[f"{m}_b1"] = _np(q["l1"]["b"]).reshape(HID, 1)
        w2 = _np(q["l2"]["w"])
        out[f"{m}_w2"] = w2
        out[f"{m}_b2"] = (_np(q["l2"]["b"]) - w2.sum(0)).reshape(HID, 1)
        w3 = _np(q["l3"]["w"])
        out[f"{m}_w3"] = w3
        out[f"{m}_b3"] = (_np(q["l3"]["b"]) - w3.sum(0)).reshape(8, 1)
    return out


def _run_op_layer_device(p, operations, resources, prec, req):
    pred_mean, succ_mean, agg_mach = _op_layer_inputs(
        operations, resources, prec, req
    )
    wmap = _wmap(p)
    in_maps = []
    for k in range(N_CORES):
        sl = slice(k * ROWS, (k + 1) * ROWS)
        m = dict(wmap)
        m["x_all"] = np.concatenate(
            [
                _pad8(pred_mean[sl].T),
                _pad8(succ_mean[sl].T),
                _pad8(agg_mach[sl].T),
                _pad8(operations[sl].T),
            ],
            axis=0,
        )
        in_maps.append(m)
    nc = _get_nc()
    trace = bool(int(os.environ.get("BASS_KERNEL_TRACE", "0")))
    res = run_bass_kernel_spmd(nc, in_maps, list(range(N_CORES)), trace=trace)
    if trace:
        _exec_times.append(res.exec_time_ns)
    out = np.concatenate(
        [res.results[k]["out"].T for k in range(N_CORES)], axis=0
    ).astype(np.float32)
    out[0] = 0.0
    out[-1] = 0.0
    return out


def kernel(operations, resources, precedence_edges, requirement_edges, actions, t, params):
    operations = _np(operations)
    resources = _np(resources)
    prec = np.asarray(precedence_edges)
    req = np.asarray(requirement_edges)
    actions = np.asarray(actions)
    def _conv(v):
        return {k: _conv(x) for k, x in v.items()} if isinstance(v, dict) else _np(v)

    pp = _conv(params)

    ops, res = operations, resources
    for l in range(2):
        res = _resource_layer(pp[f"res{l}"], res, ops, req)
        ops = _run_op_layer_device(pp[f"op{l}"], ops, res, prec, req)

    graph_state = np.concatenate([ops.mean(0), res.mean(0)], -1)
    state_value = _mlp_np(pp["critic"], graph_state, np.tanh)
    act_in = np.concatenate(
        [
            ops[actions[:, 0]],
            res[actions[:, 1]],
            np.broadcast_to(graph_state, (actions.shape[0], 2 * EMB)),
        ],
        -1,
    ).astype(np.float32)
    logits = _mlp_np(pp["actor"], act_in, np.tanh)
    ex = np.exp(logits - logits.max())
    probs = (ex / ex.sum()).astype(np.float32)
    return probs, np.asarray(state_value, dtype=np.float32)


# revision 11
# speedup vs baseline: 1.6874x; 1.0015x over previous
"""Trainium kernel for nn_HeterogeneousGAT: 2-layer heterogeneous GAT.

Device (8 NeuronCores, SPMD, row-sharded): the five per-node MLP stacks of
each operation_layer (the dominant ~48 GFLOP), in feature-major layout.
Host: graph gathers/scatter-adds, global-softmax attention (resource_layer),
pooling and actor/critic head.
"""
import os
import sys

sys.path.insert(0, "/opt/trn_rl_repo")

import numpy as np
import ml_dtypes

BF16 = ml_dtypes.bfloat16

import bass_rust
import concourse.bass as bass
import concourse.mybir as mybir
from concourse.bass_utils import run_bass_kernel_spmd
from concourse.tile import TileContext, ScopedClock

F = mybir.ActivationFunctionType
A = mybir.AluOpType
DT = mybir.dt.float32
DTB = mybir.dt.bfloat16
DTR = mybir.dt.float32r

N_OPS, N_RES, N_EDGES, N_ACT = 131072, 16384, 1048576, 4096
EMB, HID = 8, 128
N_CORES = 8
ROWS = N_OPS // N_CORES  # 16384 rows per core
CHUNK = 512
N_CHUNKS = ROWS // CHUNK

_exec_times = []

# ---------------------------------------------------------------- tile patch


def _drain_and_barrier_split(self, tick_clock, wait_clock):
    nc = self.nc
    collector = nc.sync.nop(nofuse=True)
    wait_clock.add_sem_waits(
        collector.ins, ScopedClock({None: tick_clock.global_clock})
    )
    si = collector.ins.sync_info
    waits = list(si.on_wait) if si is not None else []
    if waits:
        collector.ins.sync_info = bass_rust.SyncInfo(
            on_wait=[waits[0]], on_update=[]
        )
        for w in waits[1:]:
            n = nc.sync.nop(nofuse=True)
            n.ins.sync_info = bass_rust.SyncInfo(on_wait=[w], on_update=[])
    nc.sync.drain()
    nc.all_engine_barrier()
    assert self.sems is not None
    popped = nc._tile_sem_poison_stack.pop()
    assert popped is self._sem_poison
    nc.clear_and_free_semaphores(list(self.sems.allocated().values()))
    nc.all_engine_barrier()


TileContext._drain_and_barrier = _drain_and_barrier_split


def _split_waits(nc):
    """This walrus accepts one sync-wait per instruction (2 for
    EventSemaphore); move extras onto same-engine NOPs placed before."""
    n_new = 0
    for fn in nc.m.functions:
        for bb in fn.blocks:
            insns = bb.instructions
            out = []
            for inst in insns:
                si = inst.sync_info
                cap = 2 if isinstance(inst, mybir.InstEventSemaphore) else 1
                if si is not None and len(si.on_wait) > cap:
                    waits = list(si.on_wait)
                    keep, extra = waits[:cap], waits[cap:]
                    for w in extra:
                        n_new += 1
                        out.append(
                            mybir.InstNoOp(
                                name=f"waitnop-{n_new}-{inst.name}",
                                engine=inst.engine,
                                ins=[],
                                outs=[],
                                sync_info=bass_rust.SyncInfo(
                                    on_wait=[w], on_update=[]
                                ),
                            )
                        )
                    inst.sync_info = bass_rust.SyncInfo(
                        on_wait=keep, on_update=list(si.on_update)
                    )
                out.append(inst)
            if len(out) != len(insns):
                insns[:] = out
    return n_new


# ---------------------------------------------------------------- device nc

MLPS = ["pred", "succ", "res", "same"]  # comb concat order: preds,succs,aggm,same


def _build_nc():
    nc = bass.Bass()
    x_all = nc.declare_dram_parameter("x_all", [32, ROWS], DT, isOutput=False)
    wt = {}
    for m in MLPS + ["comb"]:
        if m == "comb":
            for j in range(4):
                wt[m, f"w1_{j}"] = nc.declare_dram_parameter(
                    f"{m}_w1_{j}", [8, HID], DT, isOutput=False
                )
        else:
            wt[m, "w1"] = nc.declare_dram_parameter(f"{m}_w1", [8, HID], DT, isOutput=False)
        wt[m, "b1"] = nc.declare_dram_parameter(f"{m}_b1", [HID, 1], DT, isOutput=False)
        wt[m, "w2"] = nc.declare_dram_parameter(f"{m}_w2", [HID, HID], DT, isOutput=False)
        wt[m, "b2"] = nc.declare_dram_parameter(f"{m}_b2", [HID, 1], DT, isOutput=False)
        wt[m, "w3"] = nc.declare_dram_parameter(f"{m}_w3", [HID, 8], DT, isOutput=False)
        wt[m, "b3"] = nc.declare_dram_parameter(f"{m}_b3", [8, 1], DT, isOutput=False)
    out = nc.declare_dram_parameter("out", [8, ROWS], DT, isOutput=True)

    with TileContext(nc) as tc:
        with (
            tc.tile_pool(name="xin", bufs=8) as xin,
            tc.tile_pool(name="wp", bufs=1) as wp,
            tc.tile_pool(name="hb", bufs=8) as hb,
            tc.tile_pool(name="ccp", bufs=4) as ccp,
            tc.tile_pool(name="op", bufs=3) as op,
            tc.tile_pool(name="ps", bufs=3, space="PSUM") as ps,
            tc.tile_pool(name="ps3", bufs=2, space="PSUM") as ps3,
        ):
            w = {}
            for m in MLPS + ["comb"]:
                keys = ["b1", "w2", "b2", "w3", "b3"] + (
                    [f"w1_{j}" for j in range(4)] if m == "comb" else ["w1"]
                )
                for k in keys:
                    dt_k = DT if k.startswith("b") else DTR
                    t = wp.tile(list(wt[m, k].shape), dt_k, tag=f"{m}{k}")
                    nc.gpsimd.dma_start(out=t[:], in_=wt[m, k][:])
                    w[m, k] = t

            def hidden(p_ap, b_tile, tag):
                ex = hb.tile([HID, CHUNK], DT, tag=f"ex")
                nc.scalar.activation(ex[:], p_ap, F.Exp, bias=b_tile[:], scale=1.0)
                r = hb.tile([HID, CHUNK], DT, tag=f"r")
                nc.vector.tensor_scalar(r[:], p_ap, b_tile[:], 0.0, A.add, A.max)
                h = hb.tile([HID, CHUNK], DTR, tag=f"h{tag}")
                nc.vector.scalar_tensor_tensor(h[:], ex[:], 1.0, r[:], A.min, A.add)
                return h

            def mlp(m, x_ap, out_ap):
                p1 = ps.tile([HID, CHUNK], DT, tag="p1")
                if m == "comb":
                    for j in range(4):
                        nc.tensor.matmul(
                            p1[:],
                            w[m, f"w1_{j}"][:],
                            x_ap[j][:],
                            start=(j == 0),
                            stop=(j == 3),
                        )
                else:
                    nc.tensor.matmul(p1[:], w[m, "w1"][:], x_ap, start=True, stop=True)
                h1 = hidden(p1[:], w[m, "b1"], "1")
                p2 = ps.tile([HID, CHUNK], DT, tag="p2")
                nc.tensor.matmul(p2[:], w[m, "w2"][:], h1[:], start=True, stop=True)
                h2 = hidden(p2[:], w[m, "b2"], "2")
                p3 = ps3.tile([8, CHUNK], DT, tag="p3")
                nc.tensor.matmul(p3[:], w[m, "w3"][:], h2[:], start=True, stop=True)
                nc.vector.tensor_scalar(out_ap, p3[:], w[m, "b3"][:], None, A.add)

            for c in range(N_CHUNKS):
                sl = slice(c * CHUNK, (c + 1) * CHUNK)
                ys = []
                for mi, m in enumerate(MLPS):
                    xt = xin.tile([8, CHUNK], DTR, tag="xin")
                    nc.gpsimd.dma_start(
                        out=xt[:], in_=x_all[8 * mi : 8 * mi + 8, sl]
                    )
                    y = ccp.tile([8, CHUNK], DTR, tag=f"y{m}")
                    mlp(m, xt[:], y[:])
                    ys.append(y)
                ot = op.tile([8, CHUNK], DT, tag="ot")
                mlp("comb", ys, ot[:])
                nc.gpsimd.dma_start(out=out[:, sl], in_=ot[:])

    _split_waits(nc)
    return nc


_NC = None


def _get_nc():
    global _NC
    if _NC is None:
        _NC = _build_nc()
    return _NC


# ---------------------------------------------------------------- host math


def _elu(x):
    return np.where(x > 0, x, np.expm1(np.minimum(x, 0.0)))


def _lrelu(x):
    return np.where(x > 0, x, 0.2 * x)


def _mlp_np(p, x, act):
    h = act(x @ p["l1"]["w"] + p["l1"]["b"])
    h = act(h @ p["l2"]["w"] + p["l2"]["b"])
    return h @ p["l3"]["w"] + p["l3"]["b"]


def _seg_sum(idx, vals, n):
    out = np.empty((n, vals.shape[1]), np.float32)
    for f in range(vals.shape[1]):
        out[:, f] = np.bincount(idx, weights=vals[:, f], minlength=n)
    return out


def _resource_layer(p, resources, operations, req):
    r = resources @ p["Wr"]
    o = operations @ p["Wo"]
    ops_e = o[req[0]]
    res_e = r[req[1]]
    a_self = _lrelu(np.concatenate([r, r], -1) @ p["a_self"])
    a_cross = _lrelu(res_e @ p["a"][:EMB] + ops_e @ p["a"][EMB:])
    cat = np.concatenate([a_self, a_cross], 0)
    ex = np.exp(cat - cat.max())
    norm = ex / ex.sum()
    ns, ncr = norm[: r.shape[0]], norm[r.shape[0] :]
    agg = _seg_sum(req[1], (ncr * ops_e).astype(np.float32), r.shape[0])
    return _elu(ns * r + agg).astype(np.float32)


def _np(x):
    return np.asarray(x, dtype=np.float32)


def _op_layer_inputs(operations, resources, prec, req):
    n = operations.shape[0]
    src, dst = prec[0], prec[1]
    agg_mach = _seg_sum(req[0], resources[req[1]], n)
    cnt_in = np.maximum(np.bincount(dst, minlength=n), 1.0)[:, None]
    cnt_out = np.maximum(np.bincount(src, minlength=n), 1.0)[:, None]
    pred_mean = _seg_sum(dst, operations[src], n) / cnt_in
    succ_mean = _seg_sum(src, operations[dst], n) / cnt_out
    return pred_mean.astype(np.float32), succ_mean.astype(np.float32), agg_mach


def _pad8(xT):
    if xT.shape[0] == 8:
        return np.ascontiguousarray(xT)
    out = np.zeros((8, xT.shape[1]), np.float32)
    out[: xT.shape[0]] = xT
    return out


def _wmap(p):
    """Flatten one op-layer's params into device weight arrays, folding the
    (elu+1) hidden shift into l2/l3 biases: device h' = elu(z)+1, so
    b' = b - colsum(W)."""
    out = {}
    for m in MLPS + ["comb"]:
        q = p[m]
        w1 = _np(q["l1"]["w"])
        if m == "comb":
            for j in range(4):
                out[f"{m}_w1_{j}"] = np.ascontiguousarray(w1[8 * j : 8 * j + 8])
        else:
            w1p = np.zeros((8, HID), np.float32)
            w1p[: w1.shape[0]] = w1
            out[f"{m}_w1"] = w1p
        out[f"{m}_b1"] = _np(q["l1"]["b"]).reshape(HID, 1)
        w2 = _np(q["l2"]["w"])
        out[f"{m}_w2"] = w2
        out[f"{m}_b2"] = (_np(q["l2"]["b"]) - w2.sum(0)).reshape(HID, 1)
        w3 = _np(q["l3"]["w"])
        out[f"{m}_w3"] = w3
        out[f"{m}_b3"] = (_np(q["l3"]["b"]) - w3.sum(0)).reshape(8, 1)
    return out


def _run_op_layer_device(p, operations, resources, prec, req):
    pred_mean, succ_mean, agg_mach = _op_layer_inputs(
        operations, resources, prec, req
    )
    wmap = _wmap(p)
    in_maps = []
    for k in range(N_CORES):
        sl = slice(k * ROWS, (k + 1) * ROWS)
        m = dict(wmap)
        m["x_all"] = np.concatenate(
            [
                _pad8(pred_mean[sl].T),
                _pad8(succ_mean[sl].T),
                _pad8(agg_mach[sl].T),
                _pad8(operations[sl].T),
            ],
            axis=0,
        )
        in_maps.append(m)
    nc = _get_nc()
    trace = bool(int(os.environ.get("BASS_KERNEL_TRACE", "0")))
    res = run_bass_kernel_spmd(nc, in_maps, list(range(N_CORES)), trace=trace)
    if trace:
        _exec_times.append(res.exec_time_ns)
    out = np.concatenate(
        [res.results[k]["out"].T for k in range(N_CORES)], axis=0
    ).astype(np.float32)
    out[0] = 0.0
    out[-1] = 0.0
    return out


def kernel(operations, resources, precedence_edges, requirement_edges, actions, t, params):
    operations = _np(operations)
    resources = _np(resources)
    prec = np.asarray(precedence_edges)
    req = np.asarray(requirement_edges)
    actions = np.asarray(actions)
    def _conv(v):
        return {k: _conv(x) for k, x in v.items()} if isinstance(v, dict) else _np(v)

    pp = _conv(params)

    ops, res = operations, resources
    for l in range(2):
        res = _resource_layer(pp[f"res{l}"], res, ops, req)
        ops = _run_op_layer_device(pp[f"op{l}"], ops, res, prec, req)

    graph_state = np.concatenate([ops.mean(0), res.mean(0)], -1)
    state_value = _mlp_np(pp["critic"], graph_state, np.tanh)
    act_in = np.concatenate(
        [
            ops[actions[:, 0]],
            res[actions[:, 1]],
            np.broadcast_to(graph_state, (actions.shape[0], 2 * EMB)),
        ],
        -1,
    ).astype(np.float32)
    logits = _mlp_np(pp["actor"], act_in, np.tanh)
    ex = np.exp(logits - logits.max())
    probs = (ex / ex.sum()).astype(np.float32)
    return probs, np.asarray(state_value, dtype=np.float32)
